# revision 2
# baseline (speedup 1.0000x reference)
"""InternLM2 decoder layer on 8 trn2 NeuronCores, tensor-parallel (bass/Tile).

Pipelined chunked collectives + DMA-transpose activation loads.

Sharding: q-heads 4c..4c+3 + kv-head c per core, wo/w2 row-sharded,
w1/w3 col-sharded. Tokens chunked 4x512; RS1/RS2 shard each 512-chunk
into 8x64 rows, so core c owns fragments {j*512+c*64..+64}.
Dataflow: norm1 -> AG1_j (token-major) -> QKV/attn/wo per chunk -> RS1_j
-> resid+norm2 (frag) -> AG2_j -> MLP (2x1024-token halves, it-major g/u,
col-sliced down-proj) -> RS2_k -> final residual. Collectives chunked and
emitted so compute overlaps them; activations bf16, psums f32.
"""
import sys
import numpy as np
import ml_dtypes

sys.path.insert(0, "/opt/trn_rl_repo")

HID, H, K, D, INTER, T = 4096, 32, 8, 128, 14336, 2048
EPS, THETA = 1e-5, 1000000.0
NC = 8                 # cores
QH = H // NC           # q heads per core = 4
JD = QH * D            # per-core attn out dim = 512
IS = INTER // NC       # inter shard = 1792
CH = 512               # token chunk
NCH = T // CH          # 4
FR = CH // NC          # fragment rows per chunk = 64
KB_ = HID // 128       # 32 k-tiles
KHALF = KB_ // 2
IT_ = IS // 128        # 14 i-tiles
HALF = 1024            # mlp token half
SCALE = 1.0 / np.sqrt(D)

bf16 = ml_dtypes.bfloat16

_compiled = None


def _build():
    from contextlib import ExitStack
    import concourse.bacc as bacc
    import concourse.bass as bass
    import concourse.tile as tile
    from concourse import mybir

    f32 = mybir.dt.float32
    bf = mybir.dt.bfloat16
    AF = mybir.ActivationFunctionType
    PSUM = bass.MemorySpace.PSUM

    nc = bacc.Bacc("TRN2", target_bir_lowering=False, debug=False, num_devices=NC)

    # ---- I/O ----
    x_own = nc.dram_tensor("x_own", [4 * FR, HID], f32, kind="ExternalInput")
    # cosF: cos duplicated to both d-halves; sinF: [-sin; +sin] (rope signs baked)
    cosT = nc.dram_tensor("cosT", [D, T], bf, kind="ExternalInput")
    sinT = nc.dram_tensor("sinT", [D, T], bf, kind="ExternalInput")
    ident = nc.dram_tensor("ident", [128, 128], bf, kind="ExternalInput")
    wqkvR = nc.dram_tensor("wqkvR", [128, KB_, JD + 2 * D], bf, kind="ExternalInput")
    woR = nc.dram_tensor("woR", [128, QH, HID], bf, kind="ExternalInput")
    w1R = nc.dram_tensor("w1R", [IT_, 128, KB_, 128], bf, kind="ExternalInput")
    w3R = nc.dram_tensor("w3R", [IT_, 128, KB_, 128], bf, kind="ExternalInput")
    w2R = nc.dram_tensor("w2R", [128, IT_, HID], bf, kind="ExternalInput")
    out_own = nc.dram_tensor("out_own", [4 * FR, HID], f32, kind="ExternalOutput")

    # ---- internal DRAM ----
    ag1_in = [nc.dram_tensor(f"ag1_in{b}", [128, HID], bf, kind="Internal")
              for b in range(2)]
    ag1_out = [nc.dram_tensor(f"ag1_out{j}", [CH, HID], bf, kind="Internal",
                              addr_space="Shared") for j in range(NCH)]
    rs1_in = [nc.dram_tensor(f"rs1_in{j}", [CH, HID], bf, kind="Internal")
              for j in range(NCH)]
    rs1_out = [nc.dram_tensor(f"rs1_out{j}", [FR, HID], bf, kind="Internal")
               for j in range(NCH)]
    ag2_in = [nc.dram_tensor(f"ag2_in{j}", [FR, HID], bf, kind="Internal")
              for j in range(NCH)]
    ag2_out = [nc.dram_tensor(f"ag2_out{j}", [CH, HID], bf, kind="Internal",
                              addr_space="Shared") for j in range(NCH)]
    rs2_in = [nc.dram_tensor(f"rs2_in{j}", [CH, HID], bf, kind="Internal")
              for j in range(NCH)]
    rs2_out = [nc.dram_tensor(f"rs2_out{j}", [FR, HID], bf, kind="Internal")
               for j in range(NCH)]
    h_spill = nc.dram_tensor("h_spill", [4 * FR, HID], f32, kind="Internal")

    RG = [list(range(NC))]

    def ag(in_ap, out_t):
        nc.gpsimd.collective_compute(
            "AllGather", mybir.AluOpType.bypass, replica_groups=RG,
            ins=[in_ap], outs=[out_t.ap()])

    def rs(in_t, out_t):
        nc.gpsimd.collective_compute(
            "ReduceScatter", mybir.AluOpType.add, replica_groups=RG,
            ins=[in_t.ap()], outs=[out_t.ap()])

    with tile.TileContext(nc) as tc, ExitStack() as top:
        const = top.enter_context(tc.tile_pool(name="const", bufs=1))
        ident_sb = const.tile([128, 128], bf)
        nc.sync.dma_start(ident_sb[:], ident.ap())
        ones_sb = const.tile([128, 1], bf)
        nc.vector.memset(ones_sb[:], 1.0)
        eps_sb = const.tile([128, 1], f32)
        nc.vector.memset(eps_sb[:], EPS)
        # causal masks for the 4 diagonal score tiles: keep iff col >= p + 128*d
        ones_wide = const.tile([128, CH], bf)
        nc.vector.memset(ones_wide[:], 1.0)
        diag_mask = const.tile([128, 4, CH], bf)
        for d_ in range(4):
            nc.gpsimd.affine_select(
                diag_mask[:, d_, :], ones_wide[:], pattern=[[1, CH]],
                compare_op=mybir.AluOpType.is_ge,
                fill=0.0, base=-128 * d_, channel_multiplier=-1)

        # attn residual + norm2 + AG2 for fragment j
        def resid_norm2(rpool, j):
            xt = rpool.tile([FR, HID], f32, tag="rxt")
            nc.sync.dma_start(xt[:], x_own.ap()[j * FR:(j + 1) * FR, :])
            rt = rpool.tile([FR, HID], bf, tag="rscr")
            nc.sync.dma_start(rt[:], rs1_out[j].ap())
            nc.vector.tensor_add(xt[:], xt[:], rt[:])   # h in place
            nc.sync.dma_start(h_spill.ap()[j * FR:(j + 1) * FR, :], xt[:])
            sq = rpool.tile([FR, HID], bf, tag="rscr")
            ssq = rpool.tile([FR, 1], f32, tag="rssq")
            nc.scalar.activation(sq[:], xt[:], AF.Square, accum_out=ssq[:])
            rms = rpool.tile([FR, 1], f32, tag="rrms")
            nc.scalar.activation(rms[:], ssq[:], AF.Sqrt,
                                 scale=1.0 / HID, bias=eps_sb[0:FR])
            rinv = rpool.tile([FR, 1], f32, tag="rrinv")
            nc.vector.reciprocal(rinv[:], rms[:])
            hn = rpool.tile([FR, HID], bf, tag="rscr")
            nc.vector.tensor_scalar_mul(hn[:], xt[:], rinv[:])
            nc.sync.dma_start(ag2_in[j].ap(), hn[:])
            ag(ag2_in[j].ap(), ag2_out[j])

        # ================= phase 1: norm1 + AG1 =================
        with ExitStack() as ph, nc.named_scope("norm1"):
            pool = ph.enter_context(tc.tile_pool(name="norm1", bufs=2))
            for b in range(2):
                xt = pool.tile([128, HID], f32, tag="xt")
                nc.sync.dma_start(xt[:], x_own.ap()[b * 128:(b + 1) * 128, :])
                sq = pool.tile([128, HID], bf, tag="sq")
                ssq = pool.tile([128, 1], f32, tag="ssq")
                nc.scalar.activation(sq[:], xt[:], AF.Square, accum_out=ssq[:])
                rms = pool.tile([128, 1], f32, tag="rms")
                nc.scalar.activation(rms[:], ssq[:], AF.Sqrt,
                                     scale=1.0 / HID, bias=eps_sb[:])
                rinv = pool.tile([128, 1], f32, tag="rinv")
                nc.vector.reciprocal(rinv[:], rms[:])
                xn = pool.tile([128, HID], bf, tag="xn")
                nc.vector.tensor_scalar_mul(xn[:], xt[:], rinv[:])
                nc.sync.dma_start(ag1_in[b].ap(), xn[:])
                for jj in range(2):
                    j = 2 * b + jj
                    ag(ag1_in[b].ap()[jj * FR:(jj + 1) * FR, :], ag1_out[j])

        # ================= phase 2: attention =================
        with ExitStack() as ph, nc.named_scope("attn"):
            # pool order chosen so the MLP pools (allocated bottom-up after
            # this scope closes) alias onto tiles that are released early
            wpool = ph.enter_context(tc.tile_pool(name="wqkv", bufs=1))
            wqkv_sb = wpool.tile([128, KB_, JD + 2 * D], bf)
            nc.sync.dma_start(wqkv_sb[:], wqkvR.ap())
            xlo_pool = ph.enter_context(tc.tile_pool(name="attnxlo", bufs=2))
            xhi_pool = ph.enter_context(tc.tile_pool(name="attnxhi", bufs=1))
            kv_pool = ph.enter_context(tc.tile_pool(name="kv", bufs=1))
            kT_sb = kv_pool.tile([128, T], bf)            # roped K, [d, t]
            v_sb = kv_pool.tile([128, T // 128, D], bf)   # [s-in-tile, s-tile, d]
            cos_sb = kv_pool.tile([D, T], bf)
            sin_sb = kv_pool.tile([D, T], bf)
            nc.sync.dma_start(cos_sb[:], cosT.ap())
            nc.sync.dma_start(sin_sb[:], sinT.ap())
            wopool = ph.enter_context(tc.tile_pool(name="wo", bufs=1))
            wo_sb = wopool.tile([128, QH, HID], bf)
            nc.sync.dma_start(wo_sb[:], woR.ap())

            sp = ph.enter_context(tc.tile_pool(name="attnsmall", bufs=1))
            ap_ = ph.enter_context(tc.tile_pool(name="attn", bufs=2))
            ap3 = ph.enter_context(tc.tile_pool(name="attn3", bufs=3))
            rpool = ph.enter_context(tc.tile_pool(name="resid", bufs=1))
            mm_ps = ph.enter_context(tc.tile_pool(name="mmps", bufs=2, space=PSUM))
            pv_ps = ph.enter_context(tc.tile_pool(name="pvps", bufs=2, space=PSUM))
            wo_ps = ph.enter_context(tc.tile_pool(name="wops", bufs=2, space=PSUM))
            KQ = KB_ // 4  # ktiles per x sub-tile = 8

            def rope(dst, src, srcswap, t0):
                # dst = src*cosF + srcswap*sinF  (sign baked into sinF)
                t1 = ap_.tile([128, CH], bf, tag="rp1")
                t2 = ap_.tile([128, CH], bf, tag="rp2")
                nc.vector.tensor_mul(t1[:], src[:], cos_sb[:, t0:t0 + CH])
                nc.vector.tensor_mul(t2[:], srcswap[:], sin_sb[:, t0:t0 + CH])
                nc.vector.tensor_add(dst, t1[:], t2[:])

            def load_xc(j):
                # 4 sub-tiles of 8 ktiles each so consumers start early
                subs = [
                    xlo_pool.tile([128, KQ, CH], bf, tag="xa", name="xa"),
                    xlo_pool.tile([128, KQ, CH], bf, tag="xb", name="xb"),
                    xhi_pool.tile([128, KQ, CH], bf, tag="xc", name="xc"),
                    xhi_pool.tile([128, KQ, CH], bf, tag="xd", name="xd"),
                ]
                for kb in range(KB_):
                    nc.sync.dma_start(
                        subs[kb // KQ][:, kb % KQ, :],
                        ag1_out[j].ap()[:, kb * 128:(kb + 1) * 128],
                        transpose=True)
                return subs

            xc_next = load_xc(0)
            for j in range(NCH):
                t0 = j * CH
                xsubs = xc_next

                if j >= 2:
                    resid_norm2(rpool, j - 2)

                def xck(kb):
                    return xsubs[kb // KQ][:, kb % KQ, :]

                qT = sp.tile([128, QH, CH], bf, tag="qT")
                for m in range(6):
                    acc = mm_ps.tile([128, CH], f32, tag="mm")
                    for kb in range(KB_):
                        nc.tensor.matmul(
                            acc[:],
                            wqkv_sb[:, kb, m * 128:(m + 1) * 128],
                            xck(kb),
                            start=(kb == 0), stop=(kb == KB_ - 1))
                    qb = ap_.tile([128, CH], bf, tag="qb")
                    nc.scalar.activation(qb[:], acc[:], AF.Copy)
                    if m <= QH:
                        qsw = ap_.tile([128, CH], bf, tag="qsw")
                        nc.scalar.activation(qsw[0:64, :], acc[64:128, :], AF.Copy)
                        nc.scalar.activation(qsw[64:128, :], acc[0:64, :], AF.Copy)
                    if m < QH:
                        rope(qT[:, m, :], qb, qsw, t0)
                    elif m == QH:
                        rope(kT_sb[:, t0:t0 + CH], qb, qsw, t0)
                    else:
                        for sb_ in range(CH // 128):
                            tp = mm_ps.tile([128, 128], bf, tag="mm")
                            nc.tensor.transpose(
                                tp[:], qb[:, sb_ * 128:(sb_ + 1) * 128],
                                ident_sb[:])
                            nc.vector.tensor_copy(
                                v_sb[:, t0 // 128 + sb_, :], tp[:])

                if j + 1 < NCH:
                    xc_next = load_xc(j + 1)

                aoT = sp.tile([128, QH, CH], bf, tag="aoT")
                for hq in range(QH):
                    pv = pv_ps.tile([128, CH], f32, tag="pv")
                    den = pv_ps.tile([1, CH], f32, tag="den")
                    ns = (t0 + CH) // 128

                    def emit_sc_exp(si):
                        # score matmul + exp (+ causal mask on diagonal tiles)
                        sc = mm_ps.tile([128, CH], f32, tag="mm")
                        nc.tensor.matmul(sc[:],
                                         kT_sb[:, si * 128:(si + 1) * 128],
                                         qT[:, hq, :], start=True, stop=True)
                        pT = ap3.tile([128, CH], bf, tag="pT")
                        nc.scalar.activation(pT[:], sc[:], AF.Exp, scale=SCALE)
                        if si >= 4 * j:              # diagonal: zero s > t
                            pm = ap3.tile([128, CH], bf, tag="pm")
                            nc.vector.tensor_mul(
                                pm[:], pT[:], diag_mask[:, si - 4 * j, :])
                            pT = pm
                        return pT

                    pT_cur = emit_sc_exp(0)
                    for si in range(ns):
                        pT_nxt = emit_sc_exp(si + 1) if si + 1 < ns else None
                        nc.tensor.matmul(pv[:], v_sb[:, si, :], pT_cur[:],
                                         start=(si == 0), stop=(si == ns - 1))
                        nc.tensor.matmul(den[:], ones_sb[:], pT_cur[:],
                                         start=(si == 0), stop=(si == ns - 1))
                        pT_cur = pT_nxt
                    rec = ap_.tile([1, CH], f32, tag="rec")
                    nc.vector.reciprocal(rec[:], den[:])
                    recb = ap_.tile([128, CH], f32, tag="recb")
                    nc.gpsimd.partition_broadcast(recb[:], rec[:])
                    nc.vector.tensor_mul(aoT[:, hq, :], pv[:], recb[:])

                # wo: out[t, hid] for this chunk
                for m in range(CH // 128):
                    for nh in range(4):
                        ob = ap_.tile([128, 1024], bf, tag="ob")
                        for n2 in range(2):
                            acc = wo_ps.tile([128, 512], f32, tag="wo")
                            for kb in range(QH):
                                nc.tensor.matmul(
                                    acc[:],
                                    aoT[:, kb, m * 128:(m + 1) * 128],
                                    wo_sb[:, kb, nh * 1024 + n2 * 512:
                                          nh * 1024 + (n2 + 1) * 512],
                                    start=(kb == 0), stop=(kb == QH - 1))
                            dst = ob[:, n2 * 512:(n2 + 1) * 512]
                            if (m + nh + n2) % 2 == 0:
                                nc.scalar.activation(dst, acc[:], AF.Copy)
                            else:
                                nc.vector.tensor_copy(dst, acc[:])
                        nc.gpsimd.dma_start(
                            rs1_in[j].ap()[m * 128:(m + 1) * 128,
                                           nh * 1024:(nh + 1) * 1024], ob[:])
                rs(rs1_in[j], rs1_out[j])

            # fragment 2 resid (RS1_2 done during chunk 3)
            resid_norm2(rpool, 2)

        # ================= phase 3: MLP =================
        with ExitStack() as ph, nc.named_scope("mlp"):
            xc2_pool = ph.enter_context(tc.tile_pool(name="mlpxc", bufs=1))
            wsp = ph.enter_context(tc.tile_pool(name="w13", bufs=2))
            act_pool = ph.enter_context(tc.tile_pool(name="act", bufs=1))
            w2_pool = ph.enter_context(tc.tile_pool(name="w2s", bufs=2))
            mp = ph.enter_context(tc.tile_pool(name="mlpsm", bufs=2))
            mp3 = ph.enter_context(tc.tile_pool(name="mlpsm3", bufs=3))
            rpool2 = ph.enter_context(tc.tile_pool(name="resid2", bufs=1))
            gu_ps = ph.enter_context(tc.tile_pool(name="gups", bufs=2, space=PSUM))
            d_ps = ph.enter_context(tc.tile_pool(name="dps", bufs=2, space=PSUM))

            # final residual fragment k (out = h + rs2_out[k]), split in
            # column halves to keep the tiles small
            def final_resid(fpool, k):
                for ch2 in range(2):
                    cols = slice(ch2 * 2048, (ch2 + 1) * 2048)
                    ht = fpool.tile([FR, 2048], f32, tag="fht")
                    nc.sync.dma_start(
                        ht[:], h_spill.ap()[k * FR:(k + 1) * FR, cols])
                    rt = fpool.tile([FR, 2048], bf, tag="frt")
                    nc.sync.dma_start(rt[:], rs2_out[k].ap()[:, cols])
                    nc.vector.tensor_add(ht[:], ht[:], rt[:])
                    nc.sync.dma_start(
                        out_own.ap()[k * FR:(k + 1) * FR, cols], ht[:])

            def load_xc2(h):
                # 4 sub-tiles of 8 ktiles; return tiles + per-DMA thunks so
                # the emission can be interleaved with other sync-queue DMAs
                subs = [xc2_pool.tile([128, KQ, HALF], bf, tag=f"x2{q}",
                                      name=f"x2{q}")
                        for q in range(4)]
                thunks = []
                for kb in range(KB_):
                    for cc in range(2):
                        def thunk(kb=kb, cc=cc):
                            nc.sync.dma_start(
                                subs[kb // KQ][:, kb % KQ,
                                               cc * CH:(cc + 1) * CH],
                                ag2_out[2 * h + cc].ap()[
                                    :, kb * 128:(kb + 1) * 128],
                                transpose=True)
                        thunks.append(thunk)
                return subs, thunks

            def load_w13(it):
                w1t = wsp.tile([128, KB_, 128], bf, tag="w1t")
                w3t = wsp.tile([128, KB_, 128], bf, tag="w3t")
                nc.sync.dma_start(w1t[:], w1R.ap()[it])
                nc.sync.dma_start(w3t[:], w3R.ap()[it])
                return w1t, w3t

            xc2_next, th = load_xc2(0)
            w13_next = load_w13(0)
            for t_ in th[:32]:
                t_()
            w13_follow = load_w13(1)
            for t_ in th[32:]:
                t_()

            for h in range(2):
                xsub2 = xc2_next
                th2 = []
                actT = act_pool.tile([128, IT_, HALF], bf, tag="actT")
                for it in range(IT_):
                    w1t, w3t = w13_next
                    w13_next = w13_follow
                    if it + 2 < IT_:
                        w13_follow = load_w13(it + 2)
                    elif h == 0:
                        w13_follow = load_w13((it + 2) % IT_)
                    for tt in range(2):
                        g = gu_ps.tile([128, CH], f32, tag="g")
                        u = gu_ps.tile([128, CH], f32, tag="u")
                        for kb in range(KB_):
                            nc.tensor.matmul(
                                g[:], w1t[:, kb, :],
                                xsub2[kb // KQ][:, kb % KQ,
                                                tt * CH:(tt + 1) * CH],
                                start=(kb == 0), stop=(kb == KB_ - 1))
                        for kb in range(KB_):
                            nc.tensor.matmul(
                                u[:], w3t[:, kb, :],
                                xsub2[kb // KQ][:, kb % KQ,
                                                tt * CH:(tt + 1) * CH],
                                start=(kb == 0), stop=(kb == KB_ - 1))
                        sg = mp.tile([128, CH], f32, tag="sg")
                        nc.scalar.activation(sg[:], g[:], AF.Silu)
                        nc.vector.tensor_mul(
                            actT[:, it, tt * CH:(tt + 1) * CH], sg[:], u[:])
                    if h == 0 and it == 2:
                        # fragment 3 resid (RS1_3 long done by now)
                        resid_norm2(rpool2, 3)
                    if h == 1 and it == 6:
                        # half-0 outputs reduced by now; finish those rows
                        final_resid(rpool2, 0)
                        final_resid(rpool2, 1)

                if h == 0:
                    xc2_follow, th2 = load_xc2(1)
                    for t_ in th2[:16]:
                        t_()
                    th2 = th2[16:]

                # down-proj in two m-groups so RS2 can start mid-phase;
                # w2 re-streamed per group (col-slice outer, m inner)
                for mg in range(2):
                    for ns_ in range(8):
                        w2t = w2_pool.tile([128, IT_, 512], bf, tag="w2t")
                        nc.sync.dma_start(
                            w2t[:], w2R.ap()[:, :, ns_ * 512:(ns_ + 1) * 512])
                        if h == 0 and th2:
                            for t_ in th2[:3]:
                                t_()
                            th2 = th2[3:]
                        for m in range(mg * 4, mg * 4 + 4):
                            acc = d_ps.tile([128, 512], f32, tag="d")
                            for it in range(IT_):
                                nc.tensor.matmul(
                                    acc[:],
                                    actT[:, it, m * 128:(m + 1) * 128],
                                    w2t[:, it, :],
                                    start=(it == 0), stop=(it == IT_ - 1))
                            ob = mp3.tile([128, 512], bf, tag="ob")
                            if (ns_ + m) % 2 == 0:
                                nc.scalar.activation(ob[:], acc[:], AF.Copy)
                            else:
                                nc.vector.tensor_copy(ob[:], acc[:])
                            nc.gpsimd.dma_start(
                                rs2_in[2 * h + mg].ap()[
                                    (m % 4) * 128:(m % 4 + 1) * 128,
                                    ns_ * 512:(ns_ + 1) * 512], ob[:])
                    rs(rs2_in[2 * h + mg], rs2_out[2 * h + mg])
                    if h == 1 and mg == 0:
                        final_resid(rpool2, 2)
                if h == 0:
                    for t_ in th2:
                        t_()
                    xc2_next = xc2_follow
            final_resid(rpool2, 3)

    nc.compile()
    return nc


def _get_compiled():
    global _compiled
    if _compiled is None:
        _compiled = _build()
    return _compiled


def _prep_inputs(inputs):
    x = np.asarray(inputs["hidden_states"], np.float32)
    pos = np.asarray(inputs["position_ids"]).astype(np.float32)
    wqkv = np.asarray(inputs["wqkv"], np.float32)
    wo = np.asarray(inputs["wo"], np.float32)
    w1 = np.asarray(inputs["w1"], np.float32)
    w3 = np.asarray(inputs["w3"], np.float32)
    w2 = np.asarray(inputs["w2"], np.float32)
    anw = np.asarray(inputs["attn_norm_w"], np.float32)
    fnw = np.asarray(inputs["ffn_norm_w"], np.float32)

    inv_freq = 1.0 / (THETA ** (np.arange(0, D, 2, dtype=np.float32) / D))
    freqs = pos[:, None] * inv_freq
    cos_h = np.cos(freqs).T          # [D//2, T]
    sin_h = np.sin(freqs).T
    cosT_np = np.ascontiguousarray(
        np.concatenate([cos_h, cos_h], axis=0).astype(bf16))
    sinT_np = np.ascontiguousarray(
        np.concatenate([-sin_h, sin_h], axis=0).astype(bf16))
    ident_np = np.ascontiguousarray(np.eye(128, dtype=bf16))

    wqkv_f = wqkv * anw[None, :]
    w1_f = w1 * fnw[None, :]
    w3_f = w3 * fnw[None, :]

    def ktile_major(wT, n):           # [HID, n] -> [128, KB_, n]
        return np.ascontiguousarray(
            wT.reshape(KB_, 128, n).transpose(1, 0, 2).astype(bf16))

    xr = x.reshape(NCH, CH, HID)
    in_maps = []
    for c in range(NC):
        qrows = np.arange(JD * c, JD * (c + 1))
        krows = H * D + np.arange(D * c, D * (c + 1))
        vrows = (H + K) * D + np.arange(D * c, D * (c + 1))
        rows = np.concatenate([qrows, krows, vrows])
        w1T = w1_f[IS * c:IS * (c + 1)].T          # [HID, IS]
        w3T = w3_f[IS * c:IS * (c + 1)].T
        in_maps.append({
            "x_own": np.ascontiguousarray(
                xr[:, c * FR:(c + 1) * FR, :].reshape(4 * FR, HID)),
            "cosT": cosT_np, "sinT": sinT_np, "ident": ident_np,
            "wqkvR": ktile_major(wqkv_f[rows].T, JD + 2 * D),
            "woR": np.ascontiguousarray(
                wo[:, JD * c:JD * (c + 1)].T.reshape(QH, 128, HID)
                .transpose(1, 0, 2).astype(bf16)),
            "w1R": np.ascontiguousarray(
                w1T.reshape(KB_, 128, IT_, 128).transpose(2, 1, 0, 3)
                .astype(bf16)),
            "w3R": np.ascontiguousarray(
                w3T.reshape(KB_, 128, IT_, 128).transpose(2, 1, 0, 3)
                .astype(bf16)),
            "w2R": np.ascontiguousarray(
                w2[:, IS * c:IS * (c + 1)].T.reshape(IT_, 128, HID)
                .transpose(1, 0, 2).astype(bf16)),
        })
    return in_maps


def run(inputs, trace=False):
    """Returns (output, BassKernelResults)."""
    from concourse import bass_utils
    nc = _get_compiled()
    in_maps = _prep_inputs(inputs)
    res = bass_utils.run_bass_kernel_spmd(
        nc, in_maps, core_ids=list(range(NC)), trace=trace)
    out = np.empty((T, HID), np.float32)
    outr = out.reshape(NCH, NC, FR, HID)
    for c in range(NC):
        outr[:, c, :, :] = res.results[c]["out_own"].reshape(NCH, FR, HID)
    return out, res


def kernel(**inputs):
    out, _ = run(inputs)
    return out
